# revision 10
# baseline (speedup 1.0000x reference)
"""CantorAttention Trainium2 kernel (8 NeuronCores) — banded single-phase.

Key ideas
---------
1. The Cantor function is monotone, so sorting BOTH queries and keys by
   Cantor coordinate makes each query's 64 routed keys (ties included)
   fall in a <=127-wide contiguous window of sorted key order. The
   routed gather + duplicate-route softmax then becomes a *banded dense
   masked attention*: per core, 256 sorted queries attend to a 384-wide
   key window (3 blocks of 128) with a multiplicity mask M.
2. The +-1 neighbor smoothing is linear and commutes with the k/v
   projection, so it is applied to x on the HOST (exact, f32):
   k~ = W_k^T (A x) — the device never smooths.
3. Sequence sharding (256 queries x all 8 heads per core) makes the
   output projection local: one NEFF, no collective, no second phase.

Schedule: input DMAs are split into blocks across the sync/scalar/
gpsimd queues so the q-projection starts ~1us in; the attention loop is
software-pipelined one head ahead; the normalization + output
projection is pipelined per head-pair behind the attention loop, so the
post-AV tail is only the last pair's chain.
"""
import sys

sys.path.insert(0, "/opt/trn_rl_repo")

import numpy as np
import ml_dtypes

import concourse.bass as bass
import concourse.bacc as bacc
import concourse.mybir as mybir
from concourse import tile
from concourse import bass_utils

BF16 = mybir.dt.bfloat16
F32 = mybir.dt.float32
Exp = mybir.ActivationFunctionType.Exp

S = 2048
D = 512
H = 8
HD = 64
NCORES = 8
SS = S // NCORES  # 256 queries per core
W = 384  # key-window width per core (3 blocks of 128)
NJB = W // 128

_nc = None


def _cantor_coords(seq_len, depth=8):
    x = np.arange(seq_len, dtype=np.float64) / max(1, seq_len - 1)
    x = np.clip(x, 1e-06, 1.0 - 1e-06)
    c = np.zeros_like(x)
    factor = 0.5
    for _ in range(depth):
        xs = x * 3.0
        digit = xs.astype(np.int64)
        x = xs - digit
        c = c + (digit == 2).astype(np.float64) * factor
        factor *= 0.5
    return np.clip(c, 0.0, 1.0)


def _build(has_bv, has_bo):
    nc = bacc.Bacc("TRN2", target_bir_lowering=False, debug=False, num_devices=NCORES)
    xq_d = nc.dram_tensor("xq", [128, 4 * SS], BF16, kind="ExternalInput").ap()
    xs_d = nc.dram_tensor("xs", [128, 4 * W], BF16, kind="ExternalInput").ap()
    wq_d = nc.dram_tensor("wq", [128, 2048], BF16, kind="ExternalInput").ap()
    wk_d = nc.dram_tensor("wk", [128, 2048], BF16, kind="ExternalInput").ap()
    wv_d = nc.dram_tensor("wv", [128, 2048], BF16, kind="ExternalInput").ap()
    wo_d = nc.dram_tensor("wo", [128, 2048], BF16, kind="ExternalInput").ap()
    m_d = nc.dram_tensor("m", [128, NJB * SS], BF16, kind="ExternalInput").ap()
    sel_d = nc.dram_tensor("sel", [2, 128], BF16, kind="ExternalInput").ap()
    bq_d = nc.dram_tensor("bq", [128, 4], F32, kind="ExternalInput").ap()
    bk_d = nc.dram_tensor("bk", [128, 4], F32, kind="ExternalInput").ap()
    bv_d = nc.dram_tensor("bv", [1, 512], BF16, kind="ExternalInput").ap()
    bo_d = nc.dram_tensor("bo", [1, 512], BF16, kind="ExternalInput").ap()
    y_d = nc.dram_tensor("y", [SS, D], F32, kind="ExternalOutput").ap()

    with tile.TileContext(nc) as tc:
        with (
            tc.tile_pool(name="const", bufs=1) as const,
            tc.tile_pool(name="work", bufs=1) as work,
            tc.tile_pool(name="estream", bufs=3) as estream,
            tc.tile_pool(name="ps_proj", bufs=2, space="PSUM") as ps_proj,
            tc.tile_pool(name="ps_zd", bufs=2, space="PSUM") as ps_zd,
            tc.tile_pool(name="ps_o", bufs=2, space="PSUM") as ps_o,
        ):
            xq = const.tile([128, 4 * SS], BF16)
            xs = const.tile([128, 4 * W], BF16)
            wq = const.tile([128, 2048], BF16)
            wk = const.tile([128, 2048], BF16)
            wv = const.tile([128, 2048], BF16)
            wo = const.tile([128, 2048], BF16)
            m = const.tile([128, NJB * SS], BF16)
            sel = const.tile([2, 128], BF16)
            bq = const.tile([128, 4], F32)
            bk = const.tile([128, 4], F32)
            bv = const.tile([1, 512], BF16)
            bo = const.tile([1, 512], BF16)
            ones1 = const.tile([1, 128], BF16)

            # --- input DMAs, split and interleaved so early consumers unblock fast
            nc.sync.dma_start(xq[:, 0:512], xq_d[:, 0:512])
            nc.sync.dma_start(wq[:, 0:512], wq_d[:, 0:512])
            nc.sync.dma_start(xq[:, 512:1024], xq_d[:, 512:1024])
            for hp in range(1, 4):
                nc.sync.dma_start(
                    wq[:, hp * 512 : (hp + 1) * 512], wq_d[:, hp * 512 : (hp + 1) * 512]
                )
            nc.scalar.dma_start(xs[:, 0 : 2 * W], xs_d[:, 0 : 2 * W])
            nc.scalar.dma_start(xs[:, 2 * W : 4 * W], xs_d[:, 2 * W : 4 * W])
            for hp in range(4):
                nc.scalar.dma_start(
                    wk[:, hp * 512 : (hp + 1) * 512], wk_d[:, hp * 512 : (hp + 1) * 512]
                )
            nc.scalar.dma_start(wo[:, 0:1024], wo_d[:, 0:1024])
            nc.scalar.dma_start(wo[:, 1024:2048], wo_d[:, 1024:2048])
            nc.gpsimd.dma_start(m[:], m_d[:])
            nc.gpsimd.dma_start(wv[:, 0:1024], wv_d[:, 0:1024])
            nc.gpsimd.dma_start(wv[:, 1024:2048], wv_d[:, 1024:2048])
            nc.gpsimd.dma_start(sel[:], sel_d[:])
            nc.gpsimd.dma_start(bq[:], bq_d[:])
            nc.gpsimd.dma_start(bk[:], bk_d[:])
            if has_bv:
                nc.gpsimd.dma_start(bv[:], bv_d[:])
            if has_bo:
                nc.gpsimd.dma_start(bo[:], bo_d[:])
            nc.gpsimd.memset(ones1[:], 1.0)

            qt = work.tile([128, 4 * SS], BF16)  # tile hp: [q_{2hp}|q_{2hp+1}] x queries
            kt = work.tile([128, 4 * W], BF16)  # tile hp: [k_{2hp}|k_{2hp+1}] x window
            # vjd[jb]: [128 j, 8*65]; head h cols h*65..h*65+65 = [v|1]
            vjd = [work.tile([128, 8 * 65], BF16, name=f"vjd{j}") for j in range(NJB)]
            uf = work.tile([128, 4 * SS], F32)  # u staging, hp layout
            lfE = work.tile([65, 4 * SS], F32)  # row 64: even-head l per hp block
            lap = work.tile([2, 4 * SS], F32)  # row 0: even l, row 1: odd l
            rl2 = work.tile([2, 4 * SS], F32)
            rlb = work.tile([2, 4 * SS], BF16)
            un = work.tile([128, 4 * SS], BF16)

            # ones-init vjd so the per-head 65th column is already 1
            for jb in range(NJB):
                nc.gpsimd.memset(vjd[jb][:], 1.0)

            # --- q projection (4 head-pair tiles)
            for hp in range(4):
                pp = ps_proj.tile([128, 512], F32, tag="pp")
                for c in range(4):
                    nc.tensor.matmul(
                        pp[:, 0:SS],
                        wq[:, (hp * 4 + c) * 128 : (hp * 4 + c + 1) * 128],
                        xq[:, c * SS : (c + 1) * SS],
                        start=(c == 0),
                        stop=(c == 3),
                    )
                nc.vector.tensor_scalar_add(
                    qt[:, hp * SS : (hp + 1) * SS], pp[:, 0:SS], bq[:, hp : hp + 1]
                )

            # --- k projection (4 head-pair tiles over the window)
            for hp in range(4):
                pp = ps_proj.tile([128, 512], F32, tag="pp")
                for c in range(4):
                    nc.tensor.matmul(
                        pp[:, 0:W],
                        wk[:, (hp * 4 + c) * 128 : (hp * 4 + c + 1) * 128],
                        xs[:, c * W : (c + 1) * W],
                        start=(c == 0),
                        stop=(c == 3),
                    )
                nc.vector.tensor_scalar_add(
                    kt[:, hp * W : (hp + 1) * W], pp[:, 0:W], bk[:, hp : hp + 1]
                )

            # --- v projection, direct [j, (h,d)] orientation per block
            for jb in range(NJB):
                pv = ps_proj.tile([128, 512], F32, tag="pp")
                for c in range(4):
                    nc.tensor.matmul(
                        pv[:],
                        xs[:, c * W + jb * 128 : c * W + (jb + 1) * 128],
                        wv[:, c * 512 : (c + 1) * 512],
                        start=(c == 0),
                        stop=(c == 3 and not has_bv),
                    )
                if has_bv:
                    nc.tensor.matmul(pv[:], ones1[:], bv[:], start=False, stop=True)
                for h in range(H):
                    nc.vector.tensor_copy(
                        vjd[jb][:, h * 65 : h * 65 + 64], pv[:, h * 64 : (h + 1) * 64]
                    )

            # --- attention + per-pair tail, software pipelined
            es = {}
            py = [
                ps_proj.tile([128, 512], F32, tag="pp", name=f"py{q2}") for q2 in range(2)
            ]

            def scores(h):
                hp, r = h // 2, 64 * (h % 2)
                zd = ps_zd.tile([128, NJB * SS], F32, tag="zd")
                for jb in range(NJB):
                    nc.tensor.matmul(
                        zd[:, jb * SS : (jb + 1) * SS],
                        kt[r : r + 64, hp * W + jb * 128 : hp * W + (jb + 1) * 128],
                        qt[r : r + 64, hp * SS : (hp + 1) * SS],
                        start=True,
                        stop=True,
                    )
                e = estream.tile([128, NJB * SS], BF16, tag="e")
                nc.scalar.activation(e[:], zd[:], Exp)
                nc.vector.tensor_mul(e[:], e[:], m[:])
                es[h] = e

            def av(h):
                e = es.pop(h)
                po = ps_o.tile([128, SS], F32, tag="po")
                for jb in range(NJB):
                    nc.tensor.matmul(
                        po[0:65, :],
                        vjd[jb][:, h * 65 : h * 65 + 65],
                        e[:, jb * SS : (jb + 1) * SS],
                        start=(jb == 0),
                        stop=(jb == NJB - 1),
                    )
                hp = h // 2
                cs = slice(hp * SS, (hp + 1) * SS)
                if h % 2 == 0:
                    nc.vector.tensor_copy(uf[0:64, cs], po[0:64, :])
                    nc.vector.tensor_copy(lfE[64:65, cs], po[64:65, :])
                else:
                    # partition shift 0..64 -> 64..128 is DMA-only
                    ustg = work.tile([65, SS], F32, tag="ustg", bufs=2)
                    nc.vector.tensor_copy(ustg[:], po[0:65, :])
                    nc.scalar.dma_start(uf[64:128, cs], ustg[0:64, :])
                    nc.gpsimd.dma_start(lap[1:2, cs], ustg[64:65, :])

            def tail(hp):
                cs = slice(hp * SS, (hp + 1) * SS)
                nc.gpsimd.dma_start(lap[0:1, cs], lfE[64:65, cs])
                nc.vector.reciprocal(rl2[:, cs], lap[:, cs])
                nc.vector.tensor_copy(rlb[:, cs], rl2[:, cs])
                prl = ps_o.tile([128, SS], F32, tag="po", name=f"prl{hp}")
                nc.tensor.matmul(prl[:], sel[:], rlb[:, cs], start=True, stop=True)
                nc.vector.tensor_mul(un[:, cs], uf[:, cs], prl[:])
                for q2 in range(2):
                    nc.tensor.matmul(
                        py[q2][:],
                        un[:, hp * SS + q2 * 128 : hp * SS + (q2 + 1) * 128],
                        wo[:, hp * 512 : (hp + 1) * 512],
                        start=(hp == 0),
                        stop=(hp == 3 and not has_bo),
                    )

            for h in range(H):
                scores(h)
                if h > 0:
                    av(h - 1)
                    if h % 2 == 0:
                        tail(h // 2 - 1)
            av(H - 1)
            tail(3)

            for q2 in range(2):
                if has_bo:
                    nc.tensor.matmul(py[q2][:], ones1[:], bo[:], start=False, stop=True)
                ysb = work.tile([128, 512], F32, tag="ysb", bufs=2)
                nc.vector.tensor_copy(ysb[:], py[q2][:])
                nc.sync.dma_start(y_d[q2 * 128 : (q2 + 1) * 128, :], ysb[:])
    nc.compile()
    return nc


def _to_chunked(a128xN, nchunks):
    """[128*nchunks, N] -> [128, nchunks*N] with chunk c at cols c*N."""
    n = a128xN.shape[1]
    return a128xN.reshape(nchunks, 128, n).transpose(1, 0, 2).reshape(128, nchunks * n)


def _prep(x, routes, W_qkv, b_qkv, W_out, b_out):
    x2 = np.asarray(x, dtype=np.float32).reshape(S, D)
    r = np.asarray(routes).astype(np.int64)
    Wf = np.asarray(W_qkv, dtype=np.float32)
    bf = np.asarray(b_qkv, dtype=np.float32)
    Wo = np.asarray(W_out, dtype=np.float32)
    bo = np.asarray(b_out, dtype=np.float32)

    c = _cantor_coords(S)
    perm = np.argsort(c, kind="stable")
    inv = np.empty(S, dtype=np.int64)
    inv[perm] = np.arange(S)

    # smoothed x (commutes with k/v projection)
    xs = 0.5 * x2
    xs[1:] += 0.25 * x2[:-1]
    xs[0] += 0.25 * x2[0]
    xs[:-1] += 0.25 * x2[1:]
    xs[-1] += 0.25 * x2[-1]

    xT = x2.T  # [D, S]
    xsT = xs.T

    rp = inv[r]  # [S, K] key sorted-positions, rows = original query index

    wqs = (Wf[:, 0:D] * 0.125).astype(np.float32)
    wks = Wf[:, D : 2 * D]
    wvs = Wf[:, 2 * D : 3 * D]
    bqs = bf[0:D] * 0.125
    bks = bf[D : 2 * D]
    bvs = bf[2 * D : 3 * D]

    def pack_headpair(Wm):  # [D, 512] -> [128, 2048] per (hp, c) blocks
        out = np.empty((128, 2048), dtype=np.float32)
        for hp in range(4):
            cols = np.r_[2 * hp * 64 : 2 * hp * 64 + 128]
            for cc in range(4):
                blk = Wm[cc * 128 : (cc + 1) * 128, :][:, cols]
                out[:, (hp * 4 + cc) * 128 : (hp * 4 + cc + 1) * 128] = blk
        return out.astype(ml_dtypes.bfloat16)

    wq_r = pack_headpair(wqs)
    wk_r = pack_headpair(wks)
    wv_r = _to_chunked(wvs, 4).astype(ml_dtypes.bfloat16)
    wo_r = _to_chunked(Wo, 4).astype(ml_dtypes.bfloat16)

    bq_r = np.empty((128, 4), dtype=np.float32)
    bk_r = np.empty((128, 4), dtype=np.float32)
    for hp in range(4):
        bq_r[:, hp] = bqs[2 * hp * 64 : 2 * hp * 64 + 128]
        bk_r[:, hp] = bks[2 * hp * 64 : 2 * hp * 64 + 128]
    bv_r = bvs.reshape(1, 512).astype(ml_dtypes.bfloat16)
    bo_r = bo.reshape(1, 512).astype(ml_dtypes.bfloat16)

    # sel for the 1/l broadcast: row 0 -> partitions 0..63, row 1 -> 64..127
    sel = np.zeros((2, 128), dtype=np.float32)
    sel[0, 0:64] = 1.0
    sel[1, 64:128] = 1.0
    has_bv = bool(np.any(bvs))
    has_bo = bool(np.any(bo))

    in_maps = []
    for cc in range(NCORES):
        qsel = perm[cc * SS : (cc + 1) * SS]  # original query indices, sorted order
        rq = rp[qsel]  # [SS, K] key sorted-positions
        lo, hi = int(rq.min()), int(rq.max())
        assert hi - lo + 1 <= W, f"core {cc} window {hi - lo + 1} > {W}"
        w0 = min(max(0, lo), S - W)
        ksel = perm[w0 : w0 + W]  # original key indices for the window

        xq_c = np.ascontiguousarray(xT[:, qsel])
        xs_c = np.ascontiguousarray(xsT[:, ksel])
        xq_r = _to_chunked(xq_c, 4).astype(ml_dtypes.bfloat16)
        xs_r = _to_chunked(xs_c, 4).astype(ml_dtypes.bfloat16)

        Mloc = np.zeros((W, SS), dtype=np.float32)
        np.add.at(Mloc, (rq - w0, np.arange(SS)[None, :].repeat(64, axis=0).T), 1.0)
        m_r = np.empty((128, NJB * SS), dtype=np.float32)
        for jb in range(NJB):
            m_r[:, jb * SS : (jb + 1) * SS] = Mloc[jb * 128 : (jb + 1) * 128, :]
        in_maps.append(
            {
                "xq": xq_r,
                "xs": xs_r,
                "wq": wq_r,
                "wk": wk_r,
                "wv": wv_r,
                "wo": wo_r,
                "m": m_r.astype(ml_dtypes.bfloat16),
                "sel": sel.astype(ml_dtypes.bfloat16),
                "bq": bq_r,
                "bk": bk_r,
                "bv": bv_r,
                "bo": bo_r,
            }
        )
    return in_maps, perm, has_bv, has_bo


def _run(nc, in_maps, **kw):
    return bass_utils.run_bass_kernel_spmd(nc, in_maps, list(range(NCORES)), **kw)


def kernel(x, routes, W_qkv, b_qkv, W_out, b_out, _timing=None):
    global _nc
    in_maps, perm, has_bv, has_bo = _prep(x, routes, W_qkv, b_qkv, W_out, b_out)
    if _nc is None:
        _nc = _build(has_bv, has_bo)
    r1 = _run(_nc, in_maps)
    ys = np.concatenate([r1.results[c]["y"] for c in range(NCORES)], axis=0)  # [S, D]
    out = np.empty((S, D), dtype=np.float32)
    out[perm] = ys
    if _timing is not None:
        _timing["phases"] = [("fused", _nc, in_maps)]
    return out.reshape(1, S, D).astype(np.float32)
